# revision 5
# baseline (speedup 1.0000x reference)
"""Edge-parallel GNN u_mul_v kernel for Trainium2 (8 NeuronCores).

z[e, :] = h[src[e], :] * h[dst[e], :]

Strategy: shard edges across 8 cores (100K each); h (12.8MB) replicated in
HBM as the gather table. The gather primitive is the custom SWDGE
InstDMAGatherAnt (nc.gpsimd.dma_gather): thousands of 256B rows per
instruction, but signed-int16 indices (< 32768). h is therefore addressed as
two tables (h[:32768], h[32768:]) and each core's edges are bucketed on the
host into 4 groups by (src-table, dst-table); the device processes edges in
bucketed order and the host applies the inverse permutation when unsharding
(the edge->slot assignment is part of the sharding).

Per 8192-edge tile: two dma_gathers (src on SWDGE queue 0, dst on queue 1),
one DVE multiply (in place), one contiguous HWDGE store.
"""

import numpy as np

N_NODES = 50000
N_EDGES = 800000
D = 64
N_CORES = 8
E_PER_CORE = N_EDGES // N_CORES  # 100000
L = 32768  # int16-addressable rows per gather table
NI = 8192  # edges per tile (per dma_gather call)
G = NI // 128

_cached = {}  # n_tiles_per_group -> compiled nc


def _build(tiles):
    """tiles: list of (src_hi, dst_hi, ni) per tile (ni % 128 == 0, <= NI)."""
    import concourse.bass as bass
    import concourse.tile as tile
    from concourse import bacc, mybir

    T = len(tiles)
    E_DEV = sum(t[2] for t in tiles)
    nc = bacc.Bacc(
        "TRN2",
        target_bir_lowering=False,
        debug=False,
        num_devices=N_CORES,
        num_swdge_queues=4,
    )
    h_ap = nc.dram_tensor("h", [N_NODES, D], mybir.dt.float32, kind="ExternalInput").ap()
    si_ap = nc.dram_tensor(
        "src_idx", [T, 128, NI // 16], mybir.dt.int16, kind="ExternalInput"
    ).ap()
    di_ap = nc.dram_tensor(
        "dst_idx", [T, 128, NI // 16], mybir.dt.int16, kind="ExternalInput"
    ).ap()
    z_ap = nc.dram_tensor("z", [E_DEV, D], mybir.dt.float16, kind="ExternalOutput").ap()

    tab = {0: h_ap[0:L, :], 1: h_ap[L:N_NODES, :]}

    with tile.TileContext(nc) as tc:
        with (
            tc.tile_pool(name="ix", bufs=6) as ixp,
            tc.tile_pool(name="ga", bufs=4) as gap,
            tc.tile_pool(name="gb", bufs=4) as gbp,
            tc.tile_pool(name="zt", bufs=4) as ztp,
        ):
            base = 0
            for t, (s_hi, d_hi, ni) in enumerate(tiles):
                g = ni // 128
                six = ixp.tile([128, ni // 16], mybir.dt.int16, tag="six")
                nc.sync.dma_start(six[:], si_ap[t][:, : ni // 16])
                dix = ixp.tile([128, ni // 16], mybir.dt.int16, tag="dix")
                nc.sync.dma_start(dix[:], di_ap[t][:, : ni // 16])
                ga = gap.tile([128, g, D], mybir.dt.float32, tag="ga")
                nc.gpsimd.dma_gather(
                    out_ap=ga[:],
                    in_ap=tab[s_hi],
                    idxs_ap=six[:],
                    num_idxs=ni,
                    num_idxs_reg=ni,
                    elem_size=D,
                    single_packet=False,
                    queue_num=(t % 2) * 2,
                )
                gb = gbp.tile([128, g, D], mybir.dt.float32, tag="gb")
                nc.gpsimd.dma_gather(
                    out_ap=gb[:],
                    in_ap=tab[d_hi],
                    idxs_ap=dix[:],
                    num_idxs=ni,
                    num_idxs_reg=ni,
                    elem_size=D,
                    single_packet=False,
                    queue_num=(t % 2) * 2 + 1,
                )
                zt = ztp.tile([128, g, D], mybir.dt.float16, tag="zt")
                nc.vector.tensor_mul(zt[:], ga[:], gb[:])
                # device z rows [base : base+ni): slot p*g+gg holds gathered
                # position gg*128+p; contiguous per partition (g*128B runs)
                z_view = z_ap[base : base + ni, :].rearrange(
                    "(p gd) d -> p (gd d)", p=128
                )
                nc.sync.dma_start(z_view, zt[:])
                base += ni
    nc.compile()
    return nc


def _wrap16(a):
    """[ni] int16 gather-sequence -> wrapped [128, ni//16] layout:
    position i lives at partition i%16, slot i//16, replicated x8."""
    w = a.reshape(-1, 16).T
    return np.ascontiguousarray(np.tile(w, (8, 1)))


def _prepare(src, dst):
    """Bucket each core's edges by (src-table, dst-table), sort each bucket by
    src (sequential-ish HBM reads for the src gather), build per-core packed
    int16 index tensors, the shared tile structure (with variable tail tiles),
    and the device-order -> original-edge map."""
    src = np.asarray(src).astype(np.int64)
    dst = np.asarray(dst).astype(np.int64)
    groups = []  # [core][k] -> original edge indices (global), src-sorted
    for c in range(N_CORES):
        lo, hi = c * E_PER_CORE, (c + 1) * E_PER_CORE
        s, d = src[lo:hi], dst[lo:hi]
        g = (s >= L).astype(np.int64) * 2 + (d >= L).astype(np.int64)
        glist = []
        for k in range(4):
            e = np.where(g == k)[0]
            e = e[np.argsort(s[e], kind="stable")]
            glist.append(e + lo)
        groups.append(glist)
    caps = [
        -(-max(len(groups[c][k]) for c in range(N_CORES)) // 128) * 128
        for k in range(4)
    ]
    tiles = []
    for k in range(4):
        rem = caps[k]
        while rem > 0:
            ni = min(NI, rem)
            tiles.append((k >> 1, k & 1, ni))
            rem -= ni
    T = len(tiles)
    E_DEV = sum(t[2] for t in tiles)

    tile_bases = np.cumsum([0] + [t[2] for t in tiles])
    in_maps = []
    dev_orig = np.empty((N_CORES, E_DEV), np.int64)
    for c in range(N_CORES):
        orig = np.full(E_DEV, -1, np.int64)
        pos = 0
        for k in range(4):
            e = groups[c][k]
            orig[pos : pos + len(e)] = e
            pos += caps[k]
        s_loc = src[np.maximum(orig, 0)]
        d_loc = dst[np.maximum(orig, 0)]
        si = np.zeros((T, 128, NI // 16), np.int16)
        di = np.zeros((T, 128, NI // 16), np.int16)
        for t, (s_hi, d_hi, ni) in enumerate(tiles):
            b = tile_bases[t]
            s16 = np.where(
                orig[b : b + ni] >= 0, s_loc[b : b + ni] - s_hi * L, 0
            ).astype(np.int16)
            d16 = np.where(
                orig[b : b + ni] >= 0, d_loc[b : b + ni] - d_hi * L, 0
            ).astype(np.int16)
            si[t, :, : ni // 16] = _wrap16(s16)
            di[t, :, : ni // 16] = _wrap16(d16)
            # device slot p*(ni//128)+g holds gathered position g*128+p
            tmap = np.arange(ni).reshape(ni // 128, 128).T.reshape(-1)
            dev_orig[c, b : b + ni] = orig[b : b + ni][tmap]
        in_maps.append({"si": si, "di": di})
    return tiles, in_maps, dev_orig


def _get_nc(tiles):
    key = tuple(tiles)
    if key not in _cached:
        _cached[key] = _build(list(key))
    return _cached[key]


def _make_in_maps(h, src, dst):
    tiles, idx_maps, dev_orig = _prepare(src, dst)
    h32 = np.ascontiguousarray(h, dtype=np.float32)
    in_maps = [
        {"h": h32, "src_idx": m["si"], "dst_idx": m["di"]} for m in idx_maps
    ]
    return tiles, in_maps, dev_orig


def kernel(h, src, dst):
    from concourse import bass_utils

    tiles, in_maps, dev_orig = _make_in_maps(h, src, dst)
    nc = _get_nc(tiles)
    res = bass_utils.run_bass_kernel_spmd(nc, in_maps, list(range(N_CORES)))
    out = np.empty((N_EDGES, D), np.float32)
    for c in range(N_CORES):
        zc = res.results[c]["z"]
        valid = dev_orig[c] >= 0
        out[dev_orig[c][valid]] = zc[valid].astype(np.float32)
    return out



# revision 9
# speedup vs baseline: 1.0042x; 1.0042x over previous
"""Edge-parallel GNN u_mul_v kernel for Trainium2 (8 NeuronCores).

z[e, :] = h[src[e], :] * h[dst[e], :]

Sharding: edges are globally sorted by src and cores take contiguous 100K
spans, so each core's src values live in a narrow ~6.3K-node window. That
window of h is shipped per-core as a rebased gather table (hsrc, <= 8192
rows), so src indices fit the gather's signed-int16 format with no table
split, and the src gather only touches a 1.6MB hot set of HBM.

Per core, edges are bucketed by dst table half (h[:32768] / h[32768:], the
int16 index limit) and sorted by dst inside each bucket: the dst gather then
sweeps h near-sequentially instead of randomly. Device programs are SPMD so
bucket capacities are padded to the per-bucket max across cores.

Per 8192-edge tile: two SWDGE dma_gathers (256B rows; src from hsrc on one
queue, dst from h lo/hi on another), one DVE multiply producing fp16, one
contiguous HWDGE store of z [E_DEV, 64] fp16 (host converts to fp32 and
applies the inverse edge permutation).
"""

import numpy as np

N_NODES = 50000
N_EDGES = 800000
D = 64
N_CORES = 8
E_PER_CORE = N_EDGES // N_CORES  # 100000
L = 32768  # int16-addressable rows per gather table
W_SRC = 8192  # per-core src window table rows (window is ~6.3K wide)
NI = 8192  # edges per tile (per dma_gather call)

_cached = {}  # tiles structure -> compiled nc


def _build(tiles):
    """tiles: list of (dst_hi, ni) per tile (ni % 128 == 0, <= NI)."""
    import concourse.bass as bass
    import concourse.tile as tile
    from concourse import bacc, mybir

    T = len(tiles)
    E_DEV = sum(t[1] for t in tiles)
    nc = bacc.Bacc(
        "TRN2",
        target_bir_lowering=False,
        debug=False,
        num_devices=N_CORES,
        num_swdge_queues=4,
    )
    h_ap = nc.dram_tensor("h", [N_NODES, D], mybir.dt.float32, kind="ExternalInput").ap()
    hs_ap = nc.dram_tensor(
        "hsrc", [W_SRC, D], mybir.dt.float32, kind="ExternalInput"
    ).ap()
    si_ap = nc.dram_tensor(
        "src_idx", [T, 128, NI // 16], mybir.dt.int16, kind="ExternalInput"
    ).ap()
    di_ap = nc.dram_tensor(
        "dst_idx", [T, 128, NI // 16], mybir.dt.int16, kind="ExternalInput"
    ).ap()
    z_ap = nc.dram_tensor("z", [E_DEV, D], mybir.dt.float16, kind="ExternalOutput").ap()

    dtab = {0: h_ap[0:L, :], 1: h_ap[L:N_NODES, :]}

    with tile.TileContext(nc) as tc:
        with (
            tc.tile_pool(name="ix", bufs=6) as ixp,
            tc.tile_pool(name="ga", bufs=4) as gap,
            tc.tile_pool(name="gb", bufs=4) as gbp,
            tc.tile_pool(name="zt", bufs=4) as ztp,
        ):
            base = 0
            for t, (d_hi, ni) in enumerate(tiles):
                g = ni // 128
                six = ixp.tile([128, ni // 16], mybir.dt.int16, tag="six")
                nc.sync.dma_start(six[:], si_ap[t][:, : ni // 16])
                dix = ixp.tile([128, ni // 16], mybir.dt.int16, tag="dix")
                nc.sync.dma_start(dix[:], di_ap[t][:, : ni // 16])
                ga = gap.tile([128, g, D], mybir.dt.float32, tag="ga")
                nc.gpsimd.dma_gather(
                    out_ap=ga[:],
                    in_ap=hs_ap,
                    idxs_ap=six[:],
                    num_idxs=ni,
                    num_idxs_reg=ni,
                    elem_size=D,
                    single_packet=False,
                    queue_num=(t % 2) * 2,
                )
                gb = gbp.tile([128, g, D], mybir.dt.float32, tag="gb")
                nc.gpsimd.dma_gather(
                    out_ap=gb[:],
                    in_ap=dtab[d_hi],
                    idxs_ap=dix[:],
                    num_idxs=ni,
                    num_idxs_reg=ni,
                    elem_size=D,
                    single_packet=False,
                    queue_num=(t % 2) * 2 + 1,
                )
                zt = ztp.tile([128, g, D], mybir.dt.float16, tag="zt")
                nc.vector.tensor_mul(zt[:], ga[:], gb[:])
                # device z rows [base : base+ni): slot p*g+gg holds gathered
                # position gg*128+p; contiguous per partition (g*128B runs)
                z_view = z_ap[base : base + ni, :].rearrange(
                    "(p gd) d -> p (gd d)", p=128
                )
                nc.sync.dma_start(z_view, zt[:])
                base += ni
    nc.compile()
    return nc


def _wrap16(a):
    """[ni] int16 gather-sequence -> wrapped [128, ni//16] layout:
    position i lives at partition i%16, slot i//16, replicated x8."""
    w = a.reshape(-1, 16).T
    return np.ascontiguousarray(np.tile(w, (8, 1)))


def _prepare(src, dst):
    """Globally sort edges by src; shard contiguous spans; per core bucket by
    dst-table half and sort by dst inside each bucket. Build per-core packed
    int16 index tensors (src rebased to the core's window), the shared tile
    structure, per-core window bases, and the device-order -> edge map."""
    src = np.asarray(src).astype(np.int64)
    dst = np.asarray(dst).astype(np.int64)
    order = np.argsort(src, kind="stable")
    spans = [order[c * E_PER_CORE : (c + 1) * E_PER_CORE] for c in range(N_CORES)]
    n0s = [int(src[sp].min()) for sp in spans]
    groups = []  # [core][k] -> original edge ids, dst-sorted
    for c in range(N_CORES):
        e = spans[c]
        k = (dst[e] >= L).astype(np.int64)
        glist = []
        for kk in range(2):
            ee = e[k == kk]
            ee = ee[np.argsort(dst[ee], kind="stable")]
            glist.append(ee)
        groups.append(glist)
        assert src[e].max() - n0s[c] < W_SRC
    caps = [
        -(-max(len(groups[c][k]) for c in range(N_CORES)) // 128) * 128
        for k in range(2)
    ]
    tiles = []
    for k in range(2):
        rem = caps[k]
        while rem > 0:
            ni = min(NI, rem)
            tiles.append((k, ni))
            rem -= ni
    T = len(tiles)
    E_DEV = sum(t[1] for t in tiles)

    tile_bases = np.cumsum([0] + [t[1] for t in tiles])
    in_maps = []
    dev_orig = np.empty((N_CORES, E_DEV), np.int64)
    for c in range(N_CORES):
        orig = np.full(E_DEV, -1, np.int64)
        pos = 0
        for k in range(2):
            e = groups[c][k]
            orig[pos : pos + len(e)] = e
            pos += caps[k]
        s_loc = src[np.maximum(orig, 0)] - n0s[c]
        d_loc = dst[np.maximum(orig, 0)]
        si = np.zeros((T, 128, NI // 16), np.int16)
        di = np.zeros((T, 128, NI // 16), np.int16)
        for t, (d_hi, ni) in enumerate(tiles):
            b = tile_bases[t]
            s16 = np.where(orig[b : b + ni] >= 0, s_loc[b : b + ni], 0).astype(
                np.int16
            )
            d16 = np.where(
                orig[b : b + ni] >= 0, d_loc[b : b + ni] - d_hi * L, 0
            ).astype(np.int16)
            si[t, :, : ni // 16] = _wrap16(s16)
            di[t, :, : ni // 16] = _wrap16(d16)
            # device slot p*(ni//128)+g holds gathered position g*128+p
            tmap = np.arange(ni).reshape(ni // 128, 128).T.reshape(-1)
            dev_orig[c, b : b + ni] = orig[b : b + ni][tmap]
        in_maps.append({"si": si, "di": di})
    return tiles, in_maps, dev_orig, n0s


def _get_nc(tiles):
    key = tuple(tiles)
    if key not in _cached:
        _cached[key] = _build(list(key))
    return _cached[key]


def _make_in_maps(h, src, dst):
    tiles, idx_maps, dev_orig, n0s = _prepare(src, dst)
    h32 = np.ascontiguousarray(h, dtype=np.float32)
    in_maps = []
    for c, m in enumerate(idx_maps):
        hs = np.zeros((W_SRC, D), np.float32)
        end = min(n0s[c] + W_SRC, N_NODES)
        hs[: end - n0s[c]] = h32[n0s[c] : end]
        in_maps.append(
            {"h": h32, "hsrc": hs, "src_idx": m["si"], "dst_idx": m["di"]}
        )
    return tiles, in_maps, dev_orig


def kernel(h, src, dst):
    from concourse import bass_utils

    tiles, in_maps, dev_orig = _make_in_maps(h, src, dst)
    nc = _get_nc(tiles)
    res = bass_utils.run_bass_kernel_spmd(nc, in_maps, list(range(N_CORES)))
    out = np.empty((N_EDGES, D), np.float32)
    for c in range(N_CORES):
        zc = res.results[c]["z"]
        valid = dev_orig[c] >= 0
        out[dev_orig[c][valid]] = zc[valid].astype(np.float32)
    return out


# revision 11
# speedup vs baseline: 1.7437x; 1.7363x over previous
"""Edge-parallel GNN u_mul_v kernel for Trainium2 (8 NeuronCores).

z[e, :] = h[src[e], :] * h[dst[e], :]

Sharding: edges are globally sorted by src and cores take contiguous 100K
spans, so each core's src values live in a narrow ~6.3K-node window. That
window of h is shipped per-core as a rebased gather table (hsrc, <= 8192
rows), so src indices fit the gather's signed-int16 format with no table
split, and the src gather only touches a 1.6MB hot set of HBM.

Per core, edges are bucketed by dst table half (h[:32768] / h[32768:], the
int16 index limit) and sorted by dst inside each bucket: the dst gather then
sweeps h near-sequentially instead of randomly. Device programs are SPMD so
bucket capacities are padded to the per-bucket max across cores.

Per 8192-edge tile: two SWDGE dma_gathers (256B rows; src from hsrc on one
queue, dst from h lo/hi on another), one DVE multiply producing fp16, one
contiguous HWDGE store of z [E_DEV, 64] fp16 (host converts to fp32 and
applies the inverse edge permutation).
"""

import numpy as np

N_NODES = 50000
N_EDGES = 800000
D = 64
N_CORES = 8
E_PER_CORE = N_EDGES // N_CORES  # 100000
L = 32768  # int16-addressable rows per gather table
W_SRC = 8192  # per-core src window table rows (window is ~6.3K wide)
NI = 8192  # edges per tile (per dma_gather call)

N_PAD = -(-N_NODES // 128) * 128  # 50048 table rows
N_HI = N_PAD - L  # 17280

_cached = {}  # tiles structure -> compiled nc


def _gather128(nc, out_ap, in_ap, idxs_ap, ni, queue_num):
    """Non-transpose SWDGE gather of 128-byte fp16 rows from a 256B-stride
    DRAM table. Mirrors bass.BassGpSimd.dma_gather's lowering; the wrapper's
    blanket `elem_size_bytes % 256 == 0` assert only encodes the transpose
    path's XBAR descriptor granularity — the non-transpose TX/RX descriptors
    (see q7_kernels/extended_inst/dma_gather.cpp) carry arbitrary lengths,
    only the row stride (stride_bytes_256) must be a 256B multiple."""
    from concourse import mybir

    eng = nc.gpsimd
    eng._assert_queue_num(queue_num)
    elem_size = D  # fp16 elements = 128B
    elem_step = 2 * D  # 256B row stride
    assert in_ap.dtype == out_ap.dtype == mybir.dt.float16
    assert in_ap.ap[0][0] == elem_step
    assert in_ap.ap[-1][1] == out_ap.ap[-1][1] == elem_size
    assert out_ap.ap[0][1] * out_ap.ap[1][1] == ni
    _in_ap = eng.lower_ap_dma(in_ap, for_custom_bir_dma=True)
    return eng.add_instruction(
        mybir.InstDMAGatherAnt(
            name=nc.get_next_instruction_name(),
            ins=[
                *_in_ap,
                eng.lower_ap(idxs_ap),
                eng.lower_val_access(eng.to_reg(ni)),
            ],
            outs=[eng.lower_ap(out_ap)],
            transpose=False,
            num_idxs=ni,
            elem_size=elem_size,
            stride_bytes_256=1,
            gen_mode=0,
            single_packet=False,
            queue_num=queue_num,
            sbuf_tokens_per_rank=0,
            sbuf_free_dim_per_rank=0,
            sbuf_free_dim_pad_per_rank=0,
            sbuf_byte_offset=0,
        )
    )


def _build(tiles):
    """tiles: list of (dst_hi, ni) per tile (ni % 128 == 0, <= NI)."""
    import concourse.bass as bass
    import concourse.tile as tile
    from concourse import bacc, mybir

    T = len(tiles)
    E_DEV = sum(t[1] for t in tiles)
    nc = bacc.Bacc(
        "TRN2",
        target_bir_lowering=False,
        debug=False,
        num_devices=N_CORES,
        num_swdge_queues=4,
    )
    h_ap = nc.dram_tensor(
        "h", [N_PAD, 2 * D], mybir.dt.float16, kind="ExternalInput"
    ).ap()
    hs_ap = nc.dram_tensor(
        "hsrc", [W_SRC, 2 * D], mybir.dt.float16, kind="ExternalInput"
    ).ap()
    si_ap = nc.dram_tensor(
        "src_idx", [T, 128, NI // 16], mybir.dt.int16, kind="ExternalInput"
    ).ap()
    di_ap = nc.dram_tensor(
        "dst_idx", [T, 128, NI // 16], mybir.dt.int16, kind="ExternalInput"
    ).ap()
    z_ap = nc.dram_tensor("z", [E_DEV, D], mybir.dt.float16, kind="ExternalOutput").ap()

    dtab = {0: h_ap[0:L, 0:D], 1: h_ap[L:N_PAD, 0:D]}
    stab = hs_ap[:, 0:D]

    with tile.TileContext(nc) as tc:
        with (
            tc.tile_pool(name="ix", bufs=6) as ixp,
            tc.tile_pool(name="ga", bufs=4) as gap,
            tc.tile_pool(name="gb", bufs=4) as gbp,
            tc.tile_pool(name="zt", bufs=4) as ztp,
        ):
            base = 0
            for t, (d_hi, ni) in enumerate(tiles):
                g = ni // 128
                six = ixp.tile([128, ni // 16], mybir.dt.int16, tag="six")
                nc.sync.dma_start(six[:], si_ap[t][:, : ni // 16])
                dix = ixp.tile([128, ni // 16], mybir.dt.int16, tag="dix")
                nc.sync.dma_start(dix[:], di_ap[t][:, : ni // 16])
                ga = gap.tile([128, g, D], mybir.dt.float16, tag="ga")
                _gather128(nc, ga[:], stab, six[:], ni, (t % 2) * 2)
                gb = gbp.tile([128, g, D], mybir.dt.float16, tag="gb")
                _gather128(nc, gb[:], dtab[d_hi], dix[:], ni, (t % 2) * 2 + 1)
                zt = ztp.tile([128, g, D], mybir.dt.float16, tag="zt")
                nc.vector.tensor_mul(zt[:], ga[:], gb[:])
                # device z rows [base : base+ni): slot p*g+gg holds gathered
                # position gg*128+p; contiguous per partition (g*128B runs)
                z_view = z_ap[base : base + ni, :].rearrange(
                    "(p gd) d -> p (gd d)", p=128
                )
                nc.sync.dma_start(z_view, zt[:])
                base += ni
    nc.compile()
    return nc


def _wrap16(a):
    """[ni] int16 gather-sequence -> wrapped [128, ni//16] layout:
    position i lives at partition i%16, slot i//16, replicated x8."""
    w = a.reshape(-1, 16).T
    return np.ascontiguousarray(np.tile(w, (8, 1)))


def _prepare(src, dst):
    """Globally sort edges by src; shard contiguous spans; per core bucket by
    dst-table half and sort by dst inside each bucket. Build per-core packed
    int16 index tensors (src rebased to the core's window), the shared tile
    structure, per-core window bases, and the device-order -> edge map."""
    src = np.asarray(src).astype(np.int64)
    dst = np.asarray(dst).astype(np.int64)
    order = np.argsort(src, kind="stable")
    spans = [order[c * E_PER_CORE : (c + 1) * E_PER_CORE] for c in range(N_CORES)]
    n0s = [int(src[sp].min()) for sp in spans]
    groups = []  # [core][k] -> original edge ids, dst-sorted
    for c in range(N_CORES):
        e = spans[c]
        k = (dst[e] >= L).astype(np.int64)
        glist = []
        for kk in range(2):
            ee = e[k == kk]
            ee = ee[np.argsort(dst[ee], kind="stable")]
            glist.append(ee)
        groups.append(glist)
        assert src[e].max() - n0s[c] < W_SRC
    caps = [
        -(-max(len(groups[c][k]) for c in range(N_CORES)) // 128) * 128
        for k in range(2)
    ]
    tiles = []
    for k in range(2):
        rem = caps[k]
        while rem > 0:
            ni = min(NI, rem)
            tiles.append((k, ni))
            rem -= ni
    T = len(tiles)
    E_DEV = sum(t[1] for t in tiles)

    tile_bases = np.cumsum([0] + [t[1] for t in tiles])
    in_maps = []
    dev_orig = np.empty((N_CORES, E_DEV), np.int64)
    for c in range(N_CORES):
        orig = np.full(E_DEV, -1, np.int64)
        pos = 0
        for k in range(2):
            e = groups[c][k]
            orig[pos : pos + len(e)] = e
            pos += caps[k]
        s_loc = src[np.maximum(orig, 0)] - n0s[c]
        d_loc = dst[np.maximum(orig, 0)]
        si = np.zeros((T, 128, NI // 16), np.int16)
        di = np.zeros((T, 128, NI // 16), np.int16)
        for t, (d_hi, ni) in enumerate(tiles):
            b = tile_bases[t]
            s16 = np.where(orig[b : b + ni] >= 0, s_loc[b : b + ni], 0).astype(
                np.int16
            )
            d16 = np.where(
                orig[b : b + ni] >= 0, d_loc[b : b + ni] - d_hi * L, 0
            ).astype(np.int16)
            si[t, :, : ni // 16] = _wrap16(s16)
            di[t, :, : ni // 16] = _wrap16(d16)
            # device slot p*(ni//128)+g holds gathered position g*128+p
            tmap = np.arange(ni).reshape(ni // 128, 128).T.reshape(-1)
            dev_orig[c, b : b + ni] = orig[b : b + ni][tmap]
        in_maps.append({"si": si, "di": di})
    return tiles, in_maps, dev_orig, n0s


def _get_nc(tiles):
    key = tuple(tiles)
    if key not in _cached:
        _cached[key] = _build(list(key))
    return _cached[key]


def _make_in_maps(h, src, dst):
    tiles, idx_maps, dev_orig, n0s = _prepare(src, dst)
    h16 = np.asarray(h, dtype=np.float16)
    # 256B-stride tables: row = 64 fp16 features + 64 fp16 of zero padding
    hpad = np.zeros((N_PAD, 2 * D), np.float16)
    hpad[:N_NODES, :D] = h16
    in_maps = []
    for c, m in enumerate(idx_maps):
        hs = np.zeros((W_SRC, 2 * D), np.float16)
        end = min(n0s[c] + W_SRC, N_NODES)
        hs[: end - n0s[c], :D] = h16[n0s[c] : end]
        in_maps.append(
            {"h": hpad, "hsrc": hs, "src_idx": m["si"], "dst_idx": m["di"]}
        )
    return tiles, in_maps, dev_orig


def kernel(h, src, dst):
    from concourse import bass_utils

    tiles, in_maps, dev_orig = _make_in_maps(h, src, dst)
    nc = _get_nc(tiles)
    res = bass_utils.run_bass_kernel_spmd(nc, in_maps, list(range(N_CORES)))
    out = np.empty((N_EDGES, D), np.float32)
    for c in range(N_CORES):
        zc = res.results[c]["z"]
        valid = dev_orig[c] >= 0
        out[dev_orig[c][valid]] = zc[valid].astype(np.float32)
    return out


# revision 15
# speedup vs baseline: 1.7871x; 1.0249x over previous
"""Edge-parallel GNN u_mul_v kernel for Trainium2 (8 NeuronCores).

z[e, :] = h[src[e], :] * h[dst[e], :]

Sharding: edges are globally sorted by src and cores take contiguous 100K
spans, so each core's src values live in a narrow ~6.3K-node window. That
window of h is shipped per-core as a rebased gather table (hsrc, <= 8192
rows), so src indices fit the gather's signed-int16 format with no table
split, and the src gather only touches a 1.6MB hot set of HBM.

Per core, edges are bucketed by dst table half (h[:32768] / h[32768:], the
int16 index limit) and sorted by dst inside each bucket: the dst gather then
sweeps h near-sequentially instead of randomly. Device programs are SPMD so
bucket capacities are padded to the per-bucket max across cores.

Per 8192-edge tile: two SWDGE dma_gathers (256B rows; src from hsrc on one
queue, dst from h lo/hi on another), one DVE multiply producing fp16, one
contiguous HWDGE store of z [E_DEV, 64] fp16 (host converts to fp32 and
applies the inverse edge permutation).
"""

import numpy as np

N_NODES = 50000
N_EDGES = 800000
D = 64
N_CORES = 8
E_PER_CORE = N_EDGES // N_CORES  # 100000
L = 32768  # int16-addressable rows per gather table
W_SRC = 8192  # per-core src window table rows (window is ~6.3K wide)
NI = 8192  # edges per tile (per dma_gather call)

N_PAD = -(-N_NODES // 128) * 128  # 50048 table rows
N_HI = N_PAD - L  # 17280

_cached = {}  # tiles structure -> compiled nc


def _gather128(nc, out_ap, in_ap, idxs_ap, ni, queue_num):
    """Non-transpose SWDGE gather of 128-byte fp16 rows from a 256B-stride
    DRAM table. Mirrors bass.BassGpSimd.dma_gather's lowering; the wrapper's
    blanket `elem_size_bytes % 256 == 0` assert only encodes the transpose
    path's XBAR descriptor granularity — the non-transpose TX/RX descriptors
    (see q7_kernels/extended_inst/dma_gather.cpp) carry arbitrary lengths,
    only the row stride (stride_bytes_256) must be a 256B multiple."""
    from concourse import mybir

    eng = nc.gpsimd
    eng._assert_queue_num(queue_num)
    elem_size = D  # fp16 elements = 128B
    elem_step = 2 * D  # 256B row stride
    assert in_ap.dtype == out_ap.dtype == mybir.dt.float16
    assert in_ap.ap[0][0] == elem_step
    assert in_ap.ap[-1][1] == out_ap.ap[-1][1] == elem_size
    assert out_ap.ap[0][1] * out_ap.ap[1][1] == ni
    _in_ap = eng.lower_ap_dma(in_ap, for_custom_bir_dma=True)
    return eng.add_instruction(
        mybir.InstDMAGatherAnt(
            name=nc.get_next_instruction_name(),
            ins=[
                *_in_ap,
                eng.lower_ap(idxs_ap),
                eng.lower_val_access(eng.to_reg(ni)),
            ],
            outs=[eng.lower_ap(out_ap)],
            transpose=False,
            num_idxs=ni,
            elem_size=elem_size,
            stride_bytes_256=1,
            gen_mode=0,
            single_packet=False,
            queue_num=queue_num,
            sbuf_tokens_per_rank=0,
            sbuf_free_dim_per_rank=0,
            sbuf_free_dim_pad_per_rank=0,
            sbuf_byte_offset=0,
        )
    )


def _build(tiles):
    """tiles: list of (dst_hi, ni) per tile (ni % 128 == 0, <= NI)."""
    import concourse.bass as bass
    import concourse.tile as tile
    from concourse import bacc, mybir

    T = len(tiles)
    E_DEV = sum(t[1] for t in tiles)
    nc = bacc.Bacc(
        "TRN2",
        target_bir_lowering=False,
        debug=False,
        num_devices=N_CORES,
        num_swdge_queues=4,
    )
    h_ap = nc.dram_tensor(
        "h", [N_PAD, 2 * D], mybir.dt.float16, kind="ExternalInput"
    ).ap()
    hs_ap = nc.dram_tensor(
        "hsrc", [W_SRC, 2 * D], mybir.dt.float16, kind="ExternalInput"
    ).ap()
    si_ap = nc.dram_tensor(
        "src_idx", [T, 32, NI // 16], mybir.dt.int16, kind="ExternalInput"
    ).ap()
    di_ap = nc.dram_tensor(
        "dst_idx", [T, 32, NI // 16], mybir.dt.int16, kind="ExternalInput"
    ).ap()
    z_ap = nc.dram_tensor("z", [E_DEV, D], mybir.dt.float16, kind="ExternalOutput").ap()

    dtab = {0: h_ap[0:L, 0:D], 1: h_ap[L:N_PAD, 0:D]}
    stab = hs_ap[:, 0:D]

    with tile.TileContext(nc) as tc:
        with (
            tc.tile_pool(name="ix", bufs=6) as ixp,
            tc.tile_pool(name="ga", bufs=4) as gap,
            tc.tile_pool(name="gb", bufs=4) as gbp,
            tc.tile_pool(name="zt", bufs=4) as ztp,
        ):
            base = 0
            for t, (d_hi, ni) in enumerate(tiles):
                g = ni // 128
                # each gather's Q7 pair (queue q: RX cpu 2q, TX cpu 2q+1)
                # reads idx only from partitions [32q, 32q+32) — fill just
                # those rows instead of replicating across all 128
                qs = (t % 2) * 2
                six = ixp.tile([128, ni // 16], mybir.dt.int16, tag="six")
                nc.sync.dma_start(
                    six[32 * qs : 32 * qs + 32, :], si_ap[t][:, : ni // 16]
                )
                dix = ixp.tile([128, ni // 16], mybir.dt.int16, tag="dix")
                nc.sync.dma_start(
                    dix[32 * qs + 32 : 32 * qs + 64, :], di_ap[t][:, : ni // 16]
                )
                ga = gap.tile([128, g, D], mybir.dt.float16, tag="ga")
                _gather128(nc, ga[:], stab, six[:], ni, (t % 2) * 2)
                gb = gbp.tile([128, g, D], mybir.dt.float16, tag="gb")
                _gather128(nc, gb[:], dtab[d_hi], dix[:], ni, (t % 2) * 2 + 1)
                zt = ztp.tile([128, g, D], mybir.dt.float16, tag="zt")
                nc.vector.tensor_mul(zt[:], ga[:], gb[:])
                # device z rows [base : base+ni): slot p*g+gg holds gathered
                # position gg*128+p; contiguous per partition (g*128B runs)
                z_view = z_ap[base : base + ni, :].rearrange(
                    "(p gd) d -> p (gd d)", p=128
                )
                nc.sync.dma_start(z_view, zt[:])
                base += ni
    nc.compile()
    return nc


def _wrap16(a):
    """[ni] int16 gather-sequence -> wrapped [32, ni//16] layout:
    position i lives at partition i%16, slot i//16, replicated x2 (one copy
    for each Q7 core of the queue's RX/TX pair)."""
    w = a.reshape(-1, 16).T
    return np.ascontiguousarray(np.tile(w, (2, 1)))


def _prepare(src, dst):
    """Globally sort edges by src; shard contiguous spans; per core bucket by
    dst-table half and sort by dst inside each bucket. Build per-core packed
    int16 index tensors (src rebased to the core's window), the shared tile
    structure, per-core window bases, and the device-order -> edge map."""
    src = np.asarray(src).astype(np.int64)
    dst = np.asarray(dst).astype(np.int64)
    order = np.argsort(src, kind="stable")
    spans = [order[c * E_PER_CORE : (c + 1) * E_PER_CORE] for c in range(N_CORES)]
    n0s = [int(src[sp].min()) for sp in spans]
    groups = []  # [core][k] -> original edge ids, dst-sorted
    for c in range(N_CORES):
        e = spans[c]
        k = (dst[e] >= L).astype(np.int64)
        glist = []
        for kk in range(2):
            ee = e[k == kk]
            ee = ee[np.argsort(dst[ee], kind="stable")]
            glist.append(ee)
        groups.append(glist)
        assert src[e].max() - n0s[c] < W_SRC
    caps = [
        -(-max(len(groups[c][k]) for c in range(N_CORES)) // 128) * 128
        for k in range(2)
    ]
    tiles = []
    for k in range(2):
        rem = caps[k]
        while rem > 0:
            ni = min(NI, rem)
            tiles.append((k, ni))
            rem -= ni
    T = len(tiles)
    E_DEV = sum(t[1] for t in tiles)

    tile_bases = np.cumsum([0] + [t[1] for t in tiles])
    in_maps = []
    dev_orig = np.empty((N_CORES, E_DEV), np.int64)
    for c in range(N_CORES):
        orig = np.full(E_DEV, -1, np.int64)
        pos = 0
        for k in range(2):
            e = groups[c][k]
            orig[pos : pos + len(e)] = e
            pos += caps[k]
        s_loc = src[np.maximum(orig, 0)] - n0s[c]
        d_loc = dst[np.maximum(orig, 0)]
        si = np.zeros((T, 32, NI // 16), np.int16)
        di = np.zeros((T, 32, NI // 16), np.int16)
        for t, (d_hi, ni) in enumerate(tiles):
            b = tile_bases[t]
            s16 = np.where(orig[b : b + ni] >= 0, s_loc[b : b + ni], 0).astype(
                np.int16
            )
            d16 = np.where(
                orig[b : b + ni] >= 0, d_loc[b : b + ni] - d_hi * L, 0
            ).astype(np.int16)
            si[t, :, : ni // 16] = _wrap16(s16)
            di[t, :, : ni // 16] = _wrap16(d16)
            # device slot p*(ni//128)+g holds gathered position g*128+p
            tmap = np.arange(ni).reshape(ni // 128, 128).T.reshape(-1)
            dev_orig[c, b : b + ni] = orig[b : b + ni][tmap]
        in_maps.append({"si": si, "di": di})
    return tiles, in_maps, dev_orig, n0s


def _get_nc(tiles):
    key = tuple(tiles)
    if key not in _cached:
        _cached[key] = _build(list(key))
    return _cached[key]


def _make_in_maps(h, src, dst):
    tiles, idx_maps, dev_orig, n0s = _prepare(src, dst)
    h16 = np.asarray(h, dtype=np.float16)
    # 256B-stride tables: row = 64 fp16 features + 64 fp16 of zero padding
    hpad = np.zeros((N_PAD, 2 * D), np.float16)
    hpad[:N_NODES, :D] = h16
    in_maps = []
    for c, m in enumerate(idx_maps):
        hs = np.zeros((W_SRC, 2 * D), np.float16)
        end = min(n0s[c] + W_SRC, N_NODES)
        hs[: end - n0s[c], :D] = h16[n0s[c] : end]
        in_maps.append(
            {"h": hpad, "hsrc": hs, "src_idx": m["si"], "dst_idx": m["di"]}
        )
    return tiles, in_maps, dev_orig


def kernel(h, src, dst):
    from concourse import bass_utils

    tiles, in_maps, dev_orig = _make_in_maps(h, src, dst)
    nc = _get_nc(tiles)
    res = bass_utils.run_bass_kernel_spmd(nc, in_maps, list(range(N_CORES)))
    out = np.empty((N_EDGES, D), np.float32)
    for c in range(N_CORES):
        zc = res.results[c]["z"]
        valid = dev_orig[c] >= 0
        out[dev_orig[c][valid]] = zc[valid].astype(np.float32)
    return out
